# revision 14
# baseline (speedup 1.0000x reference)
"""Trainium2 Bass kernel for nn_Attention_layer (dense_transformer).

One batch element per NeuronCore (8 cores).  Everything is laid out
hw-major: pos2 = hw*64 + d, so each hw owns 64 contiguous columns and the
whole kernel becomes a software-pipelined stream of 32 groups x 8 hw.

Per group g (8 hw, 512 cols), with pair p = (hw, hw+1), parity u = hw%2:
  A: k = wk@x, q = wq@x + bq          ([128,512] psum f32 -> bf16 sbuf)
     vT[pos2, s] = x^T @ wv^T          (transposed projection: no PE transpose
                                        of v needed; bv deferred to att copy)
  B: scoresT[j,i] = q_hw^T k_hw        (per hw, [64,64] at rows u*64)
     aT = exp(scale*scoresT)  (ACT), denom = row-sum (DVE), rcp (DVE),
     aT *= rcp  (DVE stride-0 broadcast)
  C: a = PE-transpose(aT) per parity   (bf16 psum)
     att[s, j] = vT_hw^T @ a_hw        (+ bv via ACT copy; sum_i a = 1)
  D: out = woT^T @ att  (+x+bo via DVE/Pool tensor_add with host-precomputed
     xpb = x + bo), bf16 out, DMA per 2048-col chunk.

Bias algebra: bk drops (constant in softmax axis), bq folds into q copy,
bv into att copy (softmax rows sum to 1), bo into the host-side xpb tensor.
"""

import numpy as np
import ml_dtypes

import concourse.bacc as bacc
import concourse.tile as tile
from concourse import mybir
from concourse.bass import broadcast_tensor_aps
from concourse.bass_utils import run_bass_kernel_spmd

F32 = mybir.dt.float32
BF16 = mybir.dt.bfloat16
AF = mybir.ActivationFunctionType
MUL = mybir.AluOpType.mult

B, C, S, D, H, W = 8, 256, 128, 64, 16, 16
HW = H * W              # 256
NPOS = D * HW           # 16384 (hw-major: pos2 = hw*64 + d)
SCALE = float(1.0 / np.sqrt(np.float32(S)))

NG = 32                 # groups
GHW = 8                 # hw per group
GC = GHW * D            # 512 cols per group
CH = 2048               # dma chunk cols (4 groups)
NCH = NPOS // CH        # 8

CFG = {
    "loop_n": 1,
    "skew_c": 2,        # stage C lags A by this many groups
    "bisect": 3,        # 1: A+D only, 2: A+B+D, 3: full
    "trace": False,
}

_CACHE = {}


def _emit(nc, tc, io, ctx):
    xb_d, xpb_d, wkT, wqT, wvT, woT, bq_d, bv_d, ident, out_d = io

    const = ctx.enter_context(tc.tile_pool(name="const", bufs=1))
    xr = ctx.enter_context(tc.tile_pool(name="xr", bufs=3))
    sr = ctx.enter_context(tc.tile_pool(name="sr", bufs=3))
    orb = ctx.enter_context(tc.tile_pool(name="orb", bufs=2))
    pp = ctx.enter_context(tc.tile_pool(name="pp", bufs=2, space="PSUM"))

    # ---- constants ------------------------------------------------------
    id_sb = const.tile([128, 128], BF16, tag="ident")
    nc.sync.dma_start(id_sb[:], ident[:])
    w_sb = {}
    for nm, t in (("wk", wkT), ("wq", wqT), ("wv", wvT)):
        for h in range(2):
            w_sb[nm, h] = const.tile([128, 128], BF16, tag=f"w_{nm}{h}",
                                     name=f"w_{nm}{h}")
            nc.sync.dma_start(w_sb[nm, h][:], t[h * 128:(h + 1) * 128, :])
    woT_sb = const.tile([128, 256], BF16, tag="woT")
    nc.sync.dma_start(woT_sb[:], woT[:])
    bq_sb = const.tile([128, 1], F32, tag="bq")
    nc.sync.dma_start(bq_sb[:], bq_d[:])
    bv_sb = const.tile([128, 1], F32, tag="bv")
    nc.sync.dma_start(bv_sb[:], bv_d[:])

    # pre-zero the aTT ring slots once: in-loop writes only touch the
    # diagonal quadrants, off-diagonal zeros persist across groups/iters
    for z in range(3):
        zt = sr.tile([128, 512], BF16, tag="aTT", name=f"aTTz{z}")
        nc.gpsimd.memset(zt[:], 0.0)

    loop_cm = tc.For_i(0, CFG["loop_n"], 1) if CFG["loop_n"] > 1 else None
    if loop_cm is not None:
        ctx.enter_context(loop_cm)

    xb_t, xpb_t = {}, {}
    k_t, q_t, vT_t, aT_t, aTT_t, att_t = {}, {}, {}, {}, {}, {}
    rcp_t, out_t = {}, {}

    def dma_in(ch):
        for h in range(2):
            t = xr.tile([128, CH], BF16, tag=f"xb{h}", name=f"xb{h}_{ch}")
            nc.sync.dma_start(t[:], xb_d[h * 128:(h + 1) * 128,
                                         ch * CH:(ch + 1) * CH])
            xb_t[ch, h] = t

    def dma_res(ch):
        for h in range(2):
            t = xr.tile([128, CH], BF16, tag=f"xpb{h}", name=f"xpb{h}_{ch}")
            nc.sync.dma_start(t[:], xpb_d[h * 128:(h + 1) * 128,
                                          ch * CH:(ch + 1) * CH])
            xpb_t[ch, h] = t

    def stage_a(g):
        ch, off = g // 4, (g % 4) * GC
        sl = slice(off, off + GC)
        kp = pp.tile([128, GC], F32, tag="pj", name=f"kp{g}")
        nc.tensor.matmul(kp[:], w_sb["wk", 0][:], xb_t[ch, 0][:, sl],
                         start=True, stop=False)
        nc.tensor.matmul(kp[:], w_sb["wk", 1][:], xb_t[ch, 1][:, sl],
                         start=False, stop=True)
        k_t[g] = sr.tile([128, GC], BF16, tag="k", name=f"k{g}")
        nc.scalar.copy(k_t[g][:], kp[:])
        qp = pp.tile([128, GC], F32, tag="pj", name=f"qp{g}")
        nc.tensor.matmul(qp[:], w_sb["wq", 0][:], xb_t[ch, 0][:, sl],
                         start=True, stop=False)
        nc.tensor.matmul(qp[:], w_sb["wq", 1][:], xb_t[ch, 1][:, sl],
                         start=False, stop=True)
        q_t[g] = sr.tile([128, GC], BF16, tag="q", name=f"q{g}")
        nc.scalar.activation(q_t[g][:], qp[:], AF.Identity, bias=bq_sb[:],
                             scale=1.0)
        vp = pp.tile([128, GC], F32, tag="pj", name=f"vp{g}")
        for p in range(4):
            bs = slice(off + p * 128, off + (p + 1) * 128)
            for h in range(2):
                nc.tensor.matmul(vp[:, p * 128:(p + 1) * 128],
                                 xb_t[ch, h][:, bs], w_sb["wv", h][:],
                                 start=(h == 0), stop=(h == 1),
                                 skip_group_check=True)
        vT_t[g] = sr.tile([128, GC], BF16, tag="vT", bufs=4, name=f"vT{g}")
        nc.scalar.copy(vT_t[g][:, :256], vp[:, :256])
        nc.vector.tensor_copy(vT_t[g][:, 256:], vp[:, 256:])

    def stage_b(g):
        sc = pp.tile([128, 256], F32, tag="sc", bufs=1, name=f"sc{g}")
        for p in range(4):
            for u in range(2):
                lh = 2 * p + u
                s64 = slice(lh * 64, (lh + 1) * 64)
                nc.tensor.matmul(sc[u * 64:(u + 1) * 64, p * 64:(p + 1) * 64],
                                 q_t[g][:, s64], k_t[g][:, s64],
                                 start=True, stop=True, skip_group_check=True)
        aT = sr.tile([128, 256], BF16, tag="aT", name=f"aT{g}")
        nc.scalar.activation(aT[:], sc[:], AF.Exp, scale=SCALE)
        den = sr.tile([128, 4], F32, tag="den", name=f"den{g}")
        nc.vector.reduce_sum(out=den[:],
                             in_=aT[:].rearrange("p (f i) -> p f i", i=64),
                             axis=mybir.AxisListType.X)
        rcp = sr.tile([128, 4], F32, tag="rcp", name=f"rcp{g}")
        nc.vector.reciprocal(rcp[:], den[:])
        a3 = aT[:].rearrange("p (f i) -> p f i", i=64)
        r3 = rcp[:].rearrange("p (f o) -> p f o", o=1)
        b0, b1 = broadcast_tensor_aps(a3, r3)
        nc.gpsimd.tensor_tensor(out=b0, in0=b0, in1=b1, op=MUL)
        aT_t[g] = aT

    def stage_c(g):
        # transposes grouped by parity (avoid rapid PE tile-position toggling)
        tr = pp.tile([128, 256], BF16, tag="tr", bufs=1, name=f"tr{g}")
        for u in range(2):
            rs = slice(u * 64, (u + 1) * 64)
            for p in range(4):
                nc.tensor.matmul(tr[rs, p * 64:(p + 1) * 64],
                                 aT_t[g][rs, p * 64:(p + 1) * 64],
                                 id_sb[rs, rs], is_transpose=True,
                                 start=True, stop=True, skip_group_check=True)
        # block-diagonal a per pair: [a_even 0; 0 a_odd] in a pre-zeroed tile
        aTT = sr.tile([128, 512], BF16, tag="aTT", name=f"aTT{g}")
        for u in range(2):
            rs = slice(u * 64, (u + 1) * 64)
            dst = aTT[rs, :].rearrange("p (f c) -> p f c", c=128)[
                :, :, u * 64:(u + 1) * 64]
            src = tr[rs, :].rearrange("p (f c) -> p f c", c=64)
            nc.vector.tensor_copy(dst, src)
        # att: one full-128-contract matmul per pair, tile_position (0,0)
        at = pp.tile([128, GC], F32, tag="at", name=f"at{g}")
        for p in range(4):
            nc.tensor.matmul(at[:, p * 128:(p + 1) * 128],
                             vT_t[g][:, p * 128:(p + 1) * 128],
                             aTT[:, p * 128:(p + 1) * 128],
                             start=True, stop=True, skip_group_check=True)
        att_t[g] = sr.tile([128, GC], BF16, tag="att", name=f"att{g}")
        nc.scalar.activation(att_t[g][:], at[:], AF.Identity, bias=bv_sb[:],
                             scale=1.0)

    def stage_d(g):
        ch, off = g // 4, (g % 4) * GC
        sl = slice(off, off + GC)
        if g % 4 == 0:
            for h in range(2):
                out_t[ch, h] = orb.tile([128, CH], BF16, tag=f"o{h}",
                                        name=f"o{h}_{ch}")
        for h in range(2):
            op = pp.tile([128, GC], F32, tag="op", name=f"op{g}{h}")
            nc.tensor.matmul(op[:], woT_sb[:, h * 128:(h + 1) * 128],
                             att_t[g][:], start=True, stop=True)
            nc.vector.tensor_add(out_t[ch, h][:, sl], op[:],
                                 xpb_t[ch, h][:, sl])
        if g % 4 == 3:
            for h in range(2):
                nc.sync.dma_start(out_d[h * 128:(h + 1) * 128,
                                        ch * CH:(ch + 1) * CH],
                                  out_t[ch, h][:])

    # ---- software-pipelined emission ------------------------------------
    skc = CFG["skew_c"]
    bis = CFG["bisect"]
    dma_in(0)
    dma_in(1)
    dma_res(0)
    for s in range(NG + skc + 2):
        if s < NG:
            if s % 4 == 0:
                if s // 4 + 2 < NCH:
                    dma_in(s // 4 + 2)
                if s // 4 + 1 < NCH:
                    dma_res(s // 4 + 1)
        if skc + 1 <= s < NG + skc + 1:
            stage_d(s - skc - 1)
        if skc <= s < NG + skc and bis >= 3:
            stage_c(s - skc)
        if 1 <= s <= NG and bis >= 2:
            stage_b(s - 1)
            if bis == 2:
                att_t[s - 1] = q_t[s - 1]
        if s < NG:
            stage_a(s)
            if bis == 1:
                att_t[s] = k_t[s]


def build():
    key = tuple(sorted((k, v) for k, v in CFG.items() if k != "trace"))
    if key in _CACHE:
        return _CACHE[key]
    nc = bacc.Bacc("TRN2", target_bir_lowering=False, debug=False,
                   num_devices=8)
    xb_d = nc.dram_tensor("xb", [C, NPOS], BF16, kind="ExternalInput")
    xpb_d = nc.dram_tensor("xpb", [C, NPOS], BF16, kind="ExternalInput")
    wkT = nc.dram_tensor("wkT", [C, S], BF16, kind="ExternalInput")
    wqT = nc.dram_tensor("wqT", [C, S], BF16, kind="ExternalInput")
    wvT = nc.dram_tensor("wvT", [C, S], BF16, kind="ExternalInput")
    woT = nc.dram_tensor("woT", [S, C], BF16, kind="ExternalInput")
    bq_d = nc.dram_tensor("bq", [S, 1], F32, kind="ExternalInput")
    bv_d = nc.dram_tensor("bv", [S, 1], F32, kind="ExternalInput")
    ident = nc.dram_tensor("ident", [128, 128], BF16, kind="ExternalInput")
    out_d = nc.dram_tensor("out", [C, NPOS], BF16, kind="ExternalOutput")
    from contextlib import ExitStack
    with tile.TileContext(nc) as tc, ExitStack() as ctx:
        _emit(nc, tc, (xb_d, xpb_d, wkT, wqT, wvT, woT, bq_d, bv_d, ident,
                       out_d), ctx)
    nc.compile()
    _CACHE[key] = nc
    return nc


def make_in_maps(x, wk, bk, wq, bq, wv, bv, wo, bo):
    bf = ml_dtypes.bfloat16
    x = np.asarray(x, dtype=np.float32).reshape(B, C, D, HW)
    x2 = np.ascontiguousarray(x.transpose(0, 1, 3, 2)).reshape(B, C, NPOS)
    bo = np.asarray(bo, np.float32)
    com = {
        "wkT": np.ascontiguousarray(np.asarray(wk, np.float32).T).astype(bf),
        "wqT": np.ascontiguousarray(np.asarray(wq, np.float32).T).astype(bf),
        "wvT": np.ascontiguousarray(np.asarray(wv, np.float32).T).astype(bf),
        "woT": np.ascontiguousarray(np.asarray(wo, np.float32).T).astype(bf),
        "bq": np.asarray(bq, np.float32).reshape(S, 1),
        "bv": np.asarray(bv, np.float32).reshape(S, 1),
        "ident": np.eye(128, dtype=bf),
    }
    return [dict(com, xb=x2[b].astype(bf),
                 xpb=(x2[b] + bo[:, None]).astype(bf)) for b in range(B)]


def postprocess(raw):
    """[B?, C, NPOS] hw-major bf16 -> [B, C, D, H, W] f32."""
    a = np.asarray(raw).astype(np.float32).reshape(-1, C, HW, D)
    return np.ascontiguousarray(a.transpose(0, 1, 3, 2)).reshape(-1, C, D, H, W)


def run(x, wk, bk, wq, bq, wv, bv, wo, bo, **kw):
    nc = build()
    maps = make_in_maps(x, wk, bk, wq, bq, wv, bv, wo, bo)
    res = run_bass_kernel_spmd(nc, maps, core_ids=list(range(B)), **kw)
    out = np.stack([np.asarray(r["out"]) for r in res.results])
    return postprocess(out), res


def kernel(x, wk, bk, wq, bq, wv, bv, wo, bo):
    out, _ = run(x, wk, bk, wq, bq, wv, bv, wo, bo)
    return out


# revision 15
# speedup vs baseline: 1.1309x; 1.1309x over previous
"""Trainium2 Bass kernel for nn_Attention_layer (dense_transformer).

One batch element per NeuronCore (8 cores).  Everything is laid out
hw-major: pos2 = hw*64 + d, so each hw owns 64 contiguous columns and the
whole kernel becomes a software-pipelined stream of 32 groups x 8 hw.

Per group g (8 hw, 512 cols), with pair p = (hw, hw+1), parity u = hw%2:
  A: k = wk@x, q = wq@x + bq          ([128,512] psum f32 -> bf16 sbuf)
     vT[pos2, s] = x^T @ wv^T          (transposed projection: no PE transpose
                                        of v needed; bv deferred to att copy)
  B: scoresT[j,i] = q_hw^T k_hw        (per hw, [64,64] at rows u*64)
     aT = exp(scale*scoresT)  (ACT), denom = row-sum (DVE), rcp (DVE),
     aT *= rcp  (DVE stride-0 broadcast)
  C: a = PE-transpose(aT) per parity   (bf16 psum)
     att[s, j] = vT_hw^T @ a_hw        (+ bv via ACT copy; sum_i a = 1)
  D: out = woT^T @ att  (+x+bo via DVE/Pool tensor_add with host-precomputed
     xpb = x + bo), bf16 out, DMA per 2048-col chunk.

Bias algebra: bk drops (constant in softmax axis), bq folds into q copy,
bv into att copy (softmax rows sum to 1), bo into the host-side xpb tensor.
"""

import numpy as np
import ml_dtypes

import concourse.bacc as bacc
import concourse.tile as tile
from concourse import mybir
from concourse.bass import broadcast_tensor_aps
from concourse.bass_utils import run_bass_kernel_spmd

F32 = mybir.dt.float32
BF16 = mybir.dt.bfloat16
AF = mybir.ActivationFunctionType
MUL = mybir.AluOpType.mult

B, C, S, D, H, W = 8, 256, 128, 64, 16, 16
HW = H * W              # 256
NPOS = D * HW           # 16384 (hw-major: pos2 = hw*64 + d)
SCALE = float(1.0 / np.sqrt(np.float32(S)))

NG = 32                 # groups
GHW = 8                 # hw per group
GC = GHW * D            # 512 cols per group
CH = 2048               # dma chunk cols (4 groups)
NCH = NPOS // CH        # 8

CFG = {
    "loop_n": 1,
    "lags": (2, 3, 4, 5),  # stage lags (B, C1, C2, D) behind A
    "bisect": 3,        # 1: A+D only, 2: A+B+D, 3: full
    "trace": False,
}

_CACHE = {}


def _emit(nc, tc, io, ctx):
    xb_d, xpb_d, wkT, wqT, wvT, woT, bq_d, bv_d, ident, out_d = io

    const = ctx.enter_context(tc.tile_pool(name="const", bufs=1))
    xr = ctx.enter_context(tc.tile_pool(name="xr", bufs=3))
    sr = ctx.enter_context(tc.tile_pool(name="sr", bufs=3))
    orb = ctx.enter_context(tc.tile_pool(name="orb", bufs=2))
    pp = ctx.enter_context(tc.tile_pool(name="pp", bufs=2, space="PSUM"))

    # ---- constants ------------------------------------------------------
    id_sb = const.tile([128, 128], BF16, tag="ident")
    nc.sync.dma_start(id_sb[:], ident[:])
    w_sb = {}
    for nm, t in (("wk", wkT), ("wq", wqT), ("wv", wvT)):
        for h in range(2):
            w_sb[nm, h] = const.tile([128, 128], BF16, tag=f"w_{nm}{h}",
                                     name=f"w_{nm}{h}")
            nc.sync.dma_start(w_sb[nm, h][:], t[h * 128:(h + 1) * 128, :])
    woT_sb = const.tile([128, 256], BF16, tag="woT")
    nc.sync.dma_start(woT_sb[:], woT[:])
    bq_sb = const.tile([128, 1], F32, tag="bq")
    nc.sync.dma_start(bq_sb[:], bq_d[:])
    bv_sb = const.tile([128, 1], F32, tag="bv")
    nc.sync.dma_start(bv_sb[:], bv_d[:])

    # pre-zero the aTT ring slots once: in-loop writes only touch the
    # diagonal quadrants, off-diagonal zeros persist across groups/iters
    for z in range(3):
        zt = sr.tile([128, 512], BF16, tag="aTT", name=f"aTTz{z}")
        nc.gpsimd.memset(zt[:], 0.0)

    loop_cm = tc.For_i(0, CFG["loop_n"], 1) if CFG["loop_n"] > 1 else None
    if loop_cm is not None:
        ctx.enter_context(loop_cm)

    xb_t, xpb_t = {}, {}
    k_t, q_t, vT_t, aT_t, aTT_t, att_t = {}, {}, {}, {}, {}, {}
    rcp_t, out_t = {}, {}

    def dma_in(ch):
        for h in range(2):
            t = xr.tile([128, CH], BF16, tag=f"xb{h}", name=f"xb{h}_{ch}")
            nc.sync.dma_start(t[:], xb_d[h * 128:(h + 1) * 128,
                                         ch * CH:(ch + 1) * CH])
            xb_t[ch, h] = t

    def dma_res(ch):
        for h in range(2):
            t = xr.tile([128, CH], BF16, tag=f"xpb{h}", name=f"xpb{h}_{ch}")
            nc.sync.dma_start(t[:], xpb_d[h * 128:(h + 1) * 128,
                                          ch * CH:(ch + 1) * CH])
            xpb_t[ch, h] = t

    def stage_a(g):
        ch, off = g // 4, (g % 4) * GC
        sl = slice(off, off + GC)
        kp = pp.tile([128, GC], F32, tag="pj", name=f"kp{g}")
        nc.tensor.matmul(kp[:], w_sb["wk", 0][:], xb_t[ch, 0][:, sl],
                         start=True, stop=False)
        nc.tensor.matmul(kp[:], w_sb["wk", 1][:], xb_t[ch, 1][:, sl],
                         start=False, stop=True)
        k_t[g] = sr.tile([128, GC], BF16, tag="k", bufs=4, name=f"k{g}")
        nc.scalar.copy(k_t[g][:], kp[:])
        qp = pp.tile([128, GC], F32, tag="pj", name=f"qp{g}")
        nc.tensor.matmul(qp[:], w_sb["wq", 0][:], xb_t[ch, 0][:, sl],
                         start=True, stop=False)
        nc.tensor.matmul(qp[:], w_sb["wq", 1][:], xb_t[ch, 1][:, sl],
                         start=False, stop=True)
        q_t[g] = sr.tile([128, GC], BF16, tag="q", bufs=4, name=f"q{g}")
        nc.scalar.activation(q_t[g][:], qp[:], AF.Identity, bias=bq_sb[:],
                             scale=1.0)
        vp = pp.tile([128, GC], F32, tag="pj", name=f"vp{g}")
        for p in range(4):
            bs = slice(off + p * 128, off + (p + 1) * 128)
            for h in range(2):
                nc.tensor.matmul(vp[:, p * 128:(p + 1) * 128],
                                 xb_t[ch, h][:, bs], w_sb["wv", h][:],
                                 start=(h == 0), stop=(h == 1),
                                 skip_group_check=True)
        vT_t[g] = sr.tile([128, GC], BF16, tag="vT", bufs=6, name=f"vT{g}")
        nc.scalar.copy(vT_t[g][:, :256], vp[:, :256])
        nc.vector.tensor_copy(vT_t[g][:, 256:], vp[:, 256:])

    def stage_b(g):
        sc = pp.tile([128, 256], F32, tag="sc", bufs=1, name=f"sc{g}")
        for p in range(4):
            for u in range(2):
                lh = 2 * p + u
                s64 = slice(lh * 64, (lh + 1) * 64)
                nc.tensor.matmul(sc[u * 64:(u + 1) * 64, p * 64:(p + 1) * 64],
                                 q_t[g][:, s64], k_t[g][:, s64],
                                 start=True, stop=True, skip_group_check=True)
        aT = sr.tile([128, 256], BF16, tag="aT", bufs=4, name=f"aT{g}")
        nc.scalar.activation(aT[:], sc[:], AF.Exp, scale=SCALE)
        den = sr.tile([128, 4], F32, tag="den", name=f"den{g}")
        nc.vector.reduce_sum(out=den[:],
                             in_=aT[:].rearrange("p (f i) -> p f i", i=64),
                             axis=mybir.AxisListType.X)
        rcp = sr.tile([128, 4], F32, tag="rcp", name=f"rcp{g}")
        nc.vector.reciprocal(rcp[:], den[:])
        a3 = aT[:].rearrange("p (f i) -> p f i", i=64)
        r3 = rcp[:].rearrange("p (f o) -> p f o", o=1)
        b0, b1 = broadcast_tensor_aps(a3, r3)
        nc.gpsimd.tensor_tensor(out=b0, in0=b0, in1=b1, op=MUL)
        aT_t[g] = aT

    def stage_c1(g):
        # transposes grouped by parity (avoid rapid PE tile-position toggling)
        tr = pp.tile([128, 256], BF16, tag="tr", bufs=1, name=f"tr{g}")
        for u in range(2):
            rs = slice(u * 64, (u + 1) * 64)
            for p in range(4):
                nc.tensor.matmul(tr[rs, p * 64:(p + 1) * 64],
                                 aT_t[g][rs, p * 64:(p + 1) * 64],
                                 id_sb[rs, rs], is_transpose=True,
                                 start=True, stop=True, skip_group_check=True)
        # block-diagonal a per pair: [a_even 0; 0 a_odd] in a pre-zeroed tile
        aTT = sr.tile([128, 512], BF16, tag="aTT", name=f"aTT{g}")
        for u in range(2):
            rs = slice(u * 64, (u + 1) * 64)
            dst = aTT[rs, :].rearrange("p (f c) -> p f c", c=128)[
                :, :, u * 64:(u + 1) * 64]
            src = tr[rs, :].rearrange("p (f c) -> p f c", c=64)
            nc.vector.tensor_copy(dst, src)
        aTT_t[g] = aTT

    def stage_c2(g):
        # att: one full-128-contract matmul per pair, tile_position (0,0)
        at = pp.tile([128, GC], F32, tag="at", name=f"at{g}")
        for p in range(4):
            nc.tensor.matmul(at[:, p * 128:(p + 1) * 128],
                             vT_t[g][:, p * 128:(p + 1) * 128],
                             aTT_t[g][:, p * 128:(p + 1) * 128],
                             start=True, stop=True, skip_group_check=True)
        att_t[g] = sr.tile([128, GC], BF16, tag="att", name=f"att{g}")
        nc.scalar.activation(att_t[g][:], at[:], AF.Identity, bias=bv_sb[:],
                             scale=1.0)

    def stage_d(g):
        ch, off = g // 4, (g % 4) * GC
        sl = slice(off, off + GC)
        if g % 4 == 0:
            for h in range(2):
                out_t[ch, h] = orb.tile([128, CH], BF16, tag=f"o{h}",
                                        name=f"o{h}_{ch}")
        for h in range(2):
            op = pp.tile([128, GC], F32, tag="op", name=f"op{g}{h}")
            nc.tensor.matmul(op[:], woT_sb[:, h * 128:(h + 1) * 128],
                             att_t[g][:], start=True, stop=True)
            nc.vector.tensor_add(out_t[ch, h][:, sl], op[:],
                                 xpb_t[ch, h][:, sl])
        if g % 4 == 3:
            for h in range(2):
                nc.sync.dma_start(out_d[h * 128:(h + 1) * 128,
                                        ch * CH:(ch + 1) * CH],
                                  out_t[ch, h][:])

    # ---- software-pipelined emission ------------------------------------
    LB, L1, L2, LD = CFG["lags"]
    bis = CFG["bisect"]
    dma_in(0)
    dma_in(1)
    dma_res(0)
    for s in range(NG + LD + 1):
        if s < NG:
            if s % 4 == 0:
                if s // 4 + 2 < NCH:
                    dma_in(s // 4 + 2)
                if s // 4 + 1 < NCH:
                    dma_res(s // 4 + 1)
        if LD <= s < NG + LD:
            stage_d(s - LD)
        if L2 <= s < NG + L2 and bis >= 3:
            stage_c2(s - L2)
        if L1 <= s < NG + L1 and bis >= 3:
            stage_c1(s - L1)
        if LB <= s < NG + LB and bis >= 2:
            stage_b(s - LB)
            if bis == 2:
                att_t[s - LB] = q_t[s - LB]
        if s < NG:
            stage_a(s)
            if bis == 1:
                att_t[s] = k_t[s]


def build():
    key = tuple(sorted((k, v) for k, v in CFG.items() if k != "trace"))
    if key in _CACHE:
        return _CACHE[key]
    nc = bacc.Bacc("TRN2", target_bir_lowering=False, debug=False,
                   num_devices=8)
    xb_d = nc.dram_tensor("xb", [C, NPOS], BF16, kind="ExternalInput")
    xpb_d = nc.dram_tensor("xpb", [C, NPOS], BF16, kind="ExternalInput")
    wkT = nc.dram_tensor("wkT", [C, S], BF16, kind="ExternalInput")
    wqT = nc.dram_tensor("wqT", [C, S], BF16, kind="ExternalInput")
    wvT = nc.dram_tensor("wvT", [C, S], BF16, kind="ExternalInput")
    woT = nc.dram_tensor("woT", [S, C], BF16, kind="ExternalInput")
    bq_d = nc.dram_tensor("bq", [S, 1], F32, kind="ExternalInput")
    bv_d = nc.dram_tensor("bv", [S, 1], F32, kind="ExternalInput")
    ident = nc.dram_tensor("ident", [128, 128], BF16, kind="ExternalInput")
    out_d = nc.dram_tensor("out", [C, NPOS], BF16, kind="ExternalOutput")
    from contextlib import ExitStack
    with tile.TileContext(nc) as tc, ExitStack() as ctx:
        _emit(nc, tc, (xb_d, xpb_d, wkT, wqT, wvT, woT, bq_d, bv_d, ident,
                       out_d), ctx)
    nc.compile()
    _CACHE[key] = nc
    return nc


def make_in_maps(x, wk, bk, wq, bq, wv, bv, wo, bo):
    bf = ml_dtypes.bfloat16
    x = np.asarray(x, dtype=np.float32).reshape(B, C, D, HW)
    x2 = np.ascontiguousarray(x.transpose(0, 1, 3, 2)).reshape(B, C, NPOS)
    bo = np.asarray(bo, np.float32)
    com = {
        "wkT": np.ascontiguousarray(np.asarray(wk, np.float32).T).astype(bf),
        "wqT": np.ascontiguousarray(np.asarray(wq, np.float32).T).astype(bf),
        "wvT": np.ascontiguousarray(np.asarray(wv, np.float32).T).astype(bf),
        "woT": np.ascontiguousarray(np.asarray(wo, np.float32).T).astype(bf),
        "bq": np.asarray(bq, np.float32).reshape(S, 1),
        "bv": np.asarray(bv, np.float32).reshape(S, 1),
        "ident": np.eye(128, dtype=bf),
    }
    return [dict(com, xb=x2[b].astype(bf),
                 xpb=(x2[b] + bo[:, None]).astype(bf)) for b in range(B)]


def postprocess(raw):
    """[B?, C, NPOS] hw-major bf16 -> [B, C, D, H, W] f32."""
    a = np.asarray(raw).astype(np.float32).reshape(-1, C, HW, D)
    return np.ascontiguousarray(a.transpose(0, 1, 3, 2)).reshape(-1, C, D, H, W)


def run(x, wk, bk, wq, bq, wv, bv, wo, bo, **kw):
    nc = build()
    maps = make_in_maps(x, wk, bk, wq, bq, wv, bv, wo, bo)
    res = run_bass_kernel_spmd(nc, maps, core_ids=list(range(B)), **kw)
    out = np.stack([np.asarray(r["out"]) for r in res.results])
    return postprocess(out), res


def kernel(x, wk, bk, wq, bq, wv, bv, wo, bo):
    out, _ = run(x, wk, bk, wq, bq, wv, bv, wo, bo)
    return out


# revision 16
# speedup vs baseline: 1.1582x; 1.0241x over previous
"""Trainium2 Bass kernel for nn_Attention_layer (dense_transformer).

One batch element per NeuronCore (8 cores).  Everything is laid out
hw-major: pos2 = hw*64 + d, so each hw owns 64 contiguous columns and the
whole kernel becomes a software-pipelined stream of 32 groups x 8 hw.

Per group g (8 hw, 512 cols), with pair p = (hw, hw+1), parity u = hw%2:
  A: k = wk@x, q = wq@x + bq          ([128,512] psum f32 -> bf16 sbuf)
     vT[pos2, s] = x^T @ wv^T          (transposed projection: no PE transpose
                                        of v needed; bv deferred to att copy)
  B: scoresT[j,i] = q_hw^T k_hw        (per hw, [64,64] at rows u*64)
     aT = exp(scale*scoresT)  (ACT), denom = row-sum (DVE), rcp (DVE),
     aT *= rcp  (DVE stride-0 broadcast)
  C: a = PE-transpose(aT) per parity   (bf16 psum)
     att[s, j] = vT_hw^T @ a_hw        (+ bv via ACT copy; sum_i a = 1)
  D: out = woT^T @ att  (+x+bo via DVE/Pool tensor_add with host-precomputed
     xpb = x + bo), bf16 out, DMA per 2048-col chunk.

Bias algebra: bk drops (constant in softmax axis), bq folds into q copy,
bv into att copy (softmax rows sum to 1), bo into the host-side xpb tensor.
"""

import numpy as np
import ml_dtypes

import concourse.bacc as bacc
import concourse.tile as tile
from concourse import mybir
from concourse.bass import broadcast_tensor_aps
from concourse.bass_utils import run_bass_kernel_spmd

F32 = mybir.dt.float32
BF16 = mybir.dt.bfloat16
AF = mybir.ActivationFunctionType
MUL = mybir.AluOpType.mult

B, C, S, D, H, W = 8, 256, 128, 64, 16, 16
HW = H * W              # 256
NPOS = D * HW           # 16384 (hw-major: pos2 = hw*64 + d)
SCALE = float(1.0 / np.sqrt(np.float32(S)))

NG = 32                 # groups
GHW = 8                 # hw per group
GC = GHW * D            # 512 cols per group
CH = 2048               # dma chunk cols (4 groups)
NCH = NPOS // CH        # 8

CFG = {
    "loop_n": 1,
    "lags": (2, 4, 5, 6),  # stage lags (B, C1, C2, D) behind A
    "bisect": 3,        # 1: A+D only, 2: A+B+D, 3: full
    "trace": False,
}

_CACHE = {}


def _emit(nc, tc, io, ctx):
    xb_d, xpb_d, wkT, wqT, wvT, woT, bq_d, bv_d, ident, out_d = io

    const = ctx.enter_context(tc.tile_pool(name="const", bufs=1))
    xr = ctx.enter_context(tc.tile_pool(name="xr", bufs=3))
    sr = ctx.enter_context(tc.tile_pool(name="sr", bufs=3))
    orb = ctx.enter_context(tc.tile_pool(name="orb", bufs=2))
    pp = ctx.enter_context(tc.tile_pool(name="pp", bufs=2, space="PSUM"))

    # ---- constants ------------------------------------------------------
    id_sb = const.tile([128, 128], BF16, tag="ident")
    nc.sync.dma_start(id_sb[:], ident[:])
    w_sb = {}
    for nm, t in (("wk", wkT), ("wq", wqT), ("wv", wvT)):
        for h in range(2):
            w_sb[nm, h] = const.tile([128, 128], BF16, tag=f"w_{nm}{h}",
                                     name=f"w_{nm}{h}")
            nc.sync.dma_start(w_sb[nm, h][:], t[h * 128:(h + 1) * 128, :])
    woT_sb = const.tile([128, 256], BF16, tag="woT")
    nc.sync.dma_start(woT_sb[:], woT[:])
    bq_sb = const.tile([128, 1], F32, tag="bq")
    nc.sync.dma_start(bq_sb[:], bq_d[:])
    bv_sb = const.tile([128, 1], F32, tag="bv")
    nc.sync.dma_start(bv_sb[:], bv_d[:])

    # pre-zero the aTT ring slots once: in-loop writes only touch the
    # diagonal quadrants, off-diagonal zeros persist across groups/iters
    for z in range(3):
        zt = sr.tile([128, 512], BF16, tag="aTT", name=f"aTTz{z}")
        nc.gpsimd.memset(zt[:], 0.0)

    loop_cm = tc.For_i(0, CFG["loop_n"], 1) if CFG["loop_n"] > 1 else None
    if loop_cm is not None:
        ctx.enter_context(loop_cm)

    xb_t, xpb_t = {}, {}
    k_t, q_t, vT_t, aT_t, aTT_t, att_t = {}, {}, {}, {}, {}, {}
    rcp_t, out_t = {}, {}

    def dma_in(ch):
        for h in range(2):
            t = xr.tile([128, CH], BF16, tag=f"xb{h}", name=f"xb{h}_{ch}")
            nc.sync.dma_start(t[:], xb_d[h * 128:(h + 1) * 128,
                                         ch * CH:(ch + 1) * CH])
            xb_t[ch, h] = t

    def dma_res(ch):
        for h in range(2):
            t = xr.tile([128, CH], BF16, tag=f"xpb{h}", name=f"xpb{h}_{ch}")
            nc.sync.dma_start(t[:], xpb_d[h * 128:(h + 1) * 128,
                                          ch * CH:(ch + 1) * CH])
            xpb_t[ch, h] = t

    def stage_a(g):
        ch, off = g // 4, (g % 4) * GC
        sl = slice(off, off + GC)
        kp = pp.tile([128, GC], F32, tag="pj", name=f"kp{g}")
        nc.tensor.matmul(kp[:], w_sb["wk", 0][:], xb_t[ch, 0][:, sl],
                         start=True, stop=False)
        nc.tensor.matmul(kp[:], w_sb["wk", 1][:], xb_t[ch, 1][:, sl],
                         start=False, stop=True)
        k_t[g] = sr.tile([128, GC], BF16, tag="k", bufs=4, name=f"k{g}")
        nc.scalar.copy(k_t[g][:], kp[:])
        qp = pp.tile([128, GC], F32, tag="pj", name=f"qp{g}")
        nc.tensor.matmul(qp[:], w_sb["wq", 0][:], xb_t[ch, 0][:, sl],
                         start=True, stop=False)
        nc.tensor.matmul(qp[:], w_sb["wq", 1][:], xb_t[ch, 1][:, sl],
                         start=False, stop=True)
        q_t[g] = sr.tile([128, GC], BF16, tag="q", bufs=4, name=f"q{g}")
        nc.scalar.activation(q_t[g][:], qp[:], AF.Identity, bias=bq_sb[:],
                             scale=1.0)
        vp = pp.tile([128, GC], F32, tag="pj", name=f"vp{g}")
        for p in range(4):
            bs = slice(off + p * 128, off + (p + 1) * 128)
            for h in range(2):
                nc.tensor.matmul(vp[:, p * 128:(p + 1) * 128],
                                 xb_t[ch, h][:, bs], w_sb["wv", h][:],
                                 start=(h == 0), stop=(h == 1),
                                 skip_group_check=True)
        vT_t[g] = sr.tile([128, GC], BF16, tag="vT", bufs=6, name=f"vT{g}")
        nc.scalar.copy(vT_t[g][:, :256], vp[:, :256])
        nc.vector.tensor_copy(vT_t[g][:, 256:], vp[:, 256:])

    def stage_b(g):
        sc = pp.tile([128, 256], F32, tag="sc", bufs=1, name=f"sc{g}")
        for p in range(4):
            for u in range(2):
                lh = 2 * p + u
                s64 = slice(lh * 64, (lh + 1) * 64)
                nc.tensor.matmul(sc[u * 64:(u + 1) * 64, p * 64:(p + 1) * 64],
                                 q_t[g][:, s64], k_t[g][:, s64],
                                 start=True, stop=True, skip_group_check=True)
        aT = sr.tile([128, 256], BF16, tag="aT", bufs=4, name=f"aT{g}")
        nc.scalar.activation(aT[:], sc[:], AF.Exp, scale=SCALE)
        den = sr.tile([128, 4], F32, tag="den", name=f"den{g}")
        nc.vector.reduce_sum(out=den[:],
                             in_=aT[:].rearrange("p (f i) -> p f i", i=64),
                             axis=mybir.AxisListType.X)
        rcp = sr.tile([128, 4], F32, tag="rcp", name=f"rcp{g}")
        nc.vector.reciprocal(rcp[:], den[:])
        a3 = aT[:].rearrange("p (f i) -> p f i", i=64)
        r3 = rcp[:].rearrange("p (f o) -> p f o", o=1)
        b0, b1 = broadcast_tensor_aps(a3, r3)
        nc.gpsimd.tensor_tensor(out=b0, in0=b0, in1=b1, op=MUL)
        aT_t[g] = aT

    def stage_c1(g):
        # transposes grouped by parity (avoid rapid PE tile-position toggling)
        tr = pp.tile([128, 256], BF16, tag="tr", bufs=1, name=f"tr{g}")
        for u in range(2):
            rs = slice(u * 64, (u + 1) * 64)
            for p in range(4):
                nc.tensor.matmul(tr[rs, p * 64:(p + 1) * 64],
                                 aT_t[g][rs, p * 64:(p + 1) * 64],
                                 id_sb[rs, rs], is_transpose=True,
                                 start=True, stop=True, skip_group_check=True)
        # block-diagonal a per pair: [a_even 0; 0 a_odd] in a pre-zeroed tile
        aTT = sr.tile([128, 512], BF16, tag="aTT", name=f"aTT{g}")
        for u in range(2):
            rs = slice(u * 64, (u + 1) * 64)
            dst = aTT[rs, :].rearrange("p (f c) -> p f c", c=128)[
                :, :, u * 64:(u + 1) * 64]
            src = tr[rs, :].rearrange("p (f c) -> p f c", c=64)
            nc.vector.tensor_copy(dst, src)
        aTT_t[g] = aTT

    def stage_c2(g):
        # att: one full-128-contract matmul per pair, tile_position (0,0)
        at = pp.tile([128, GC], F32, tag="at", name=f"at{g}")
        for p in range(4):
            nc.tensor.matmul(at[:, p * 128:(p + 1) * 128],
                             vT_t[g][:, p * 128:(p + 1) * 128],
                             aTT_t[g][:, p * 128:(p + 1) * 128],
                             start=True, stop=True, skip_group_check=True)
        att_t[g] = sr.tile([128, GC], BF16, tag="att", name=f"att{g}")
        nc.scalar.activation(att_t[g][:], at[:], AF.Identity, bias=bv_sb[:],
                             scale=1.0)

    def stage_d(g):
        ch, off = g // 4, (g % 4) * GC
        sl = slice(off, off + GC)
        if g % 4 == 0:
            for h in range(2):
                out_t[ch, h] = orb.tile([128, CH], BF16, tag=f"o{h}",
                                        name=f"o{h}_{ch}")
        for h in range(2):
            op = pp.tile([128, GC], F32, tag="op", name=f"op{g}{h}")
            nc.tensor.matmul(op[:], woT_sb[:, h * 128:(h + 1) * 128],
                             att_t[g][:], start=True, stop=True)
            nc.vector.tensor_add(out_t[ch, h][:, sl], op[:],
                                 xpb_t[ch, h][:, sl])
        if g % 4 == 3:
            for h in range(2):
                nc.gpsimd.dma_start(out_d[h * 128:(h + 1) * 128,
                                          ch * CH:(ch + 1) * CH],
                                    out_t[ch, h][:])

    # ---- software-pipelined emission ------------------------------------
    LB, L1, L2, LD = CFG["lags"]
    bis = CFG["bisect"]
    dma_in(0)
    dma_in(1)
    dma_res(0)
    for s in range(NG + LD + 1):
        if s < NG:
            if s % 4 == 0:
                if s // 4 + 2 < NCH:
                    dma_in(s // 4 + 2)
                if s // 4 + 1 < NCH:
                    dma_res(s // 4 + 1)
        if LD <= s < NG + LD:
            stage_d(s - LD)
        if L2 <= s < NG + L2 and bis >= 3:
            stage_c2(s - L2)
        if L1 <= s < NG + L1 and bis >= 3:
            stage_c1(s - L1)
        if LB <= s < NG + LB and bis >= 2:
            stage_b(s - LB)
            if bis == 2:
                att_t[s - LB] = q_t[s - LB]
        if s < NG:
            stage_a(s)
            if bis == 1:
                att_t[s] = k_t[s]


def build():
    key = tuple(sorted((k, v) for k, v in CFG.items() if k != "trace"))
    if key in _CACHE:
        return _CACHE[key]
    nc = bacc.Bacc("TRN2", target_bir_lowering=False, debug=False,
                   num_devices=8)
    xb_d = nc.dram_tensor("xb", [C, NPOS], BF16, kind="ExternalInput")
    xpb_d = nc.dram_tensor("xpb", [C, NPOS], BF16, kind="ExternalInput")
    wkT = nc.dram_tensor("wkT", [C, S], BF16, kind="ExternalInput")
    wqT = nc.dram_tensor("wqT", [C, S], BF16, kind="ExternalInput")
    wvT = nc.dram_tensor("wvT", [C, S], BF16, kind="ExternalInput")
    woT = nc.dram_tensor("woT", [S, C], BF16, kind="ExternalInput")
    bq_d = nc.dram_tensor("bq", [S, 1], F32, kind="ExternalInput")
    bv_d = nc.dram_tensor("bv", [S, 1], F32, kind="ExternalInput")
    ident = nc.dram_tensor("ident", [128, 128], BF16, kind="ExternalInput")
    out_d = nc.dram_tensor("out", [C, NPOS], BF16, kind="ExternalOutput")
    from contextlib import ExitStack
    with tile.TileContext(nc) as tc, ExitStack() as ctx:
        _emit(nc, tc, (xb_d, xpb_d, wkT, wqT, wvT, woT, bq_d, bv_d, ident,
                       out_d), ctx)
    nc.compile()
    _CACHE[key] = nc
    return nc


def make_in_maps(x, wk, bk, wq, bq, wv, bv, wo, bo):
    bf = ml_dtypes.bfloat16
    x = np.asarray(x, dtype=np.float32).reshape(B, C, D, HW)
    x2 = np.ascontiguousarray(x.transpose(0, 1, 3, 2)).reshape(B, C, NPOS)
    bo = np.asarray(bo, np.float32)
    com = {
        "wkT": np.ascontiguousarray(np.asarray(wk, np.float32).T).astype(bf),
        "wqT": np.ascontiguousarray(np.asarray(wq, np.float32).T).astype(bf),
        "wvT": np.ascontiguousarray(np.asarray(wv, np.float32).T).astype(bf),
        "woT": np.ascontiguousarray(np.asarray(wo, np.float32).T).astype(bf),
        "bq": np.asarray(bq, np.float32).reshape(S, 1),
        "bv": np.asarray(bv, np.float32).reshape(S, 1),
        "ident": np.eye(128, dtype=bf),
    }
    return [dict(com, xb=x2[b].astype(bf),
                 xpb=(x2[b] + bo[:, None]).astype(bf)) for b in range(B)]


def postprocess(raw):
    """[B?, C, NPOS] hw-major bf16 -> [B, C, D, H, W] f32."""
    a = np.asarray(raw).astype(np.float32).reshape(-1, C, HW, D)
    return np.ascontiguousarray(a.transpose(0, 1, 3, 2)).reshape(-1, C, D, H, W)


def run(x, wk, bk, wq, bq, wv, bv, wo, bo, **kw):
    nc = build()
    maps = make_in_maps(x, wk, bk, wq, bq, wv, bv, wo, bo)
    res = run_bass_kernel_spmd(nc, maps, core_ids=list(range(B)), **kw)
    out = np.stack([np.asarray(r["out"]) for r in res.results])
    return postprocess(out), res


def kernel(x, wk, bk, wq, bq, wv, bv, wo, bo):
    out, _ = run(x, wk, bk, wq, bq, wv, bv, wo, bo)
    return out


# revision 23
# speedup vs baseline: 1.1623x; 1.0035x over previous
"""Trainium2 Bass kernel for nn_Attention_layer (dense_transformer).

One batch element per NeuronCore (8 cores).  Everything is laid out
hw-major: pos2 = hw*64 + d, so each hw owns 64 contiguous columns and the
whole kernel becomes a software-pipelined stream of 32 groups x 8 hw.

Per group g (8 hw, 512 cols), with pair p = (hw, hw+1), parity u = hw%2:
  A: k = wk@x, q = wq@x + bq          ([128,512] psum f32 -> bf16 sbuf)
     vT[pos2, s] = x^T @ wv^T          (transposed projection: no PE transpose
                                        of v needed; bv deferred to att copy)
  B: scoresT[j,i] = q_hw^T k_hw        (per hw, [64,64] at rows u*64)
     aT = exp(scale*scoresT)  (ACT), denom = row-sum (DVE), rcp (DVE),
     aT *= rcp  (Pool, stride-0 broadcast in1)
  C: a = PE-transpose(aT) per parity   (bf16 psum)
     att[s, j] = vT_hw^T @ a_hw        (+ bv via ACT copy; sum_i a = 1)
  D: out = (woT^T @ att + bo) + x via one DVE scalar_tensor_tensor per
     half (bias and residual fused), bf16 out, DMA per 2048-col chunk.

Bias algebra: bk drops (constant in softmax axis), bq folds into q copy,
bv into att copy (softmax rows sum to 1), bo into the residual STT op.
"""

import numpy as np
import ml_dtypes

import concourse.bacc as bacc
import concourse.tile as tile
from concourse import mybir
from concourse.bass import broadcast_tensor_aps
from concourse.bass_utils import run_bass_kernel_spmd

F32 = mybir.dt.float32
BF16 = mybir.dt.bfloat16
AF = mybir.ActivationFunctionType
MUL = mybir.AluOpType.mult

B, C, S, D, H, W = 8, 256, 128, 64, 16, 16
HW = H * W              # 256
NPOS = D * HW           # 16384 (hw-major: pos2 = hw*64 + d)
SCALE = float(1.0 / np.sqrt(np.float32(S)))

NG = 32                 # groups
GHW = 8                 # hw per group
GC = GHW * D            # 512 cols per group
CH = 2048               # dma chunk cols (4 groups)
NCH = NPOS // CH        # 8

CFG = {
    "loop_n": 1,
    "lags": (2, 3, 4, 5),  # stage lags (B, C1, C2, D) behind A
    "pbufs": (2, 1, 1, 2, 2),  # psum ring depths (pj, sc, tr, at, op)
    "bisect": 3,        # 1: A+D only, 2: A+B+D, 3: full
    "trace": False,
}

_CACHE = {}


def _emit(nc, tc, io, ctx):
    xb_d, wkT, wqT, wvT, woT, bq_d, bv_d, bo_d, ident, out_d = io
    PB = CFG["pbufs"]

    const = ctx.enter_context(tc.tile_pool(name="const", bufs=1))
    xr = ctx.enter_context(tc.tile_pool(name="xr", bufs=3))
    sr = ctx.enter_context(tc.tile_pool(name="sr", bufs=3))
    orb = ctx.enter_context(tc.tile_pool(name="orb", bufs=2))
    pp = ctx.enter_context(tc.tile_pool(name="pp", bufs=2, space="PSUM"))

    # ---- constants ------------------------------------------------------
    id_sb = const.tile([128, 128], BF16, tag="ident")
    nc.sync.dma_start(id_sb[:], ident[:])
    w_sb = {}
    for nm, t in (("wk", wkT), ("wq", wqT), ("wv", wvT)):
        for h in range(2):
            w_sb[nm, h] = const.tile([128, 128], BF16, tag=f"w_{nm}{h}",
                                     name=f"w_{nm}{h}")
            nc.sync.dma_start(w_sb[nm, h][:], t[h * 128:(h + 1) * 128, :])
    woT_sb = const.tile([128, 256], BF16, tag="woT")
    nc.sync.dma_start(woT_sb[:], woT[:])
    bq_sb = const.tile([128, 1], F32, tag="bq")
    nc.sync.dma_start(bq_sb[:], bq_d[:])
    bv_sb = const.tile([128, 1], F32, tag="bv")
    nc.sync.dma_start(bv_sb[:], bv_d[:])
    bo_sb = {}
    for h in range(2):
        bo_sb[h] = const.tile([128, 1], F32, tag=f"bo{h}", name=f"bo{h}")
        nc.sync.dma_start(bo_sb[h][:], bo_d[h * 128:(h + 1) * 128, :])

    # pre-zero the aTT ring slots once: in-loop writes only touch the
    # diagonal quadrants, off-diagonal zeros persist across groups/iters
    for z in range(3):
        zt = sr.tile([128, 512], BF16, tag="aTT", name=f"aTTz{z}")
        nc.gpsimd.memset(zt[:], 0.0)

    loop_cm = tc.For_i(0, CFG["loop_n"], 1) if CFG["loop_n"] > 1 else None
    if loop_cm is not None:
        ctx.enter_context(loop_cm)

    xb_t = {}
    k_t, q_t, vT_t, aT_t, aTT_t, att_t = {}, {}, {}, {}, {}, {}
    rcp_t, out_t = {}, {}

    def dma_in(ch):
        for h in range(2):
            t = xr.tile([128, CH], BF16, tag=f"xb{h}", bufs=6, name=f"xb{h}_{ch}")
            nc.sync.dma_start(t[:], xb_d[h * 128:(h + 1) * 128,
                                         ch * CH:(ch + 1) * CH])
            xb_t[ch, h] = t

    def stage_a(g):
        ch, off = g // 4, (g % 4) * GC
        sl = slice(off, off + GC)
        kp = pp.tile([128, GC], F32, tag="pj", bufs=PB[0], name=f"kp{g}")
        nc.tensor.matmul(kp[:], w_sb["wk", 0][:], xb_t[ch, 0][:, sl],
                         start=True, stop=False)
        nc.tensor.matmul(kp[:], w_sb["wk", 1][:], xb_t[ch, 1][:, sl],
                         start=False, stop=True)
        k_t[g] = sr.tile([128, GC], BF16, tag="k", bufs=4, name=f"k{g}")
        nc.scalar.copy(k_t[g][:], kp[:])
        qp = pp.tile([128, GC], F32, tag="pj", bufs=PB[0], name=f"qp{g}")
        nc.tensor.matmul(qp[:], w_sb["wq", 0][:], xb_t[ch, 0][:, sl],
                         start=True, stop=False)
        nc.tensor.matmul(qp[:], w_sb["wq", 1][:], xb_t[ch, 1][:, sl],
                         start=False, stop=True)
        q_t[g] = sr.tile([128, GC], BF16, tag="q", bufs=4, name=f"q{g}")
        nc.scalar.activation(q_t[g][:], qp[:], AF.Identity, bias=bq_sb[:],
                             scale=1.0)
        vp = pp.tile([128, GC], F32, tag="pj", bufs=PB[0], name=f"vp{g}")
        for p in range(4):
            bs = slice(off + p * 128, off + (p + 1) * 128)
            for h in range(2):
                nc.tensor.matmul(vp[:, p * 128:(p + 1) * 128],
                                 xb_t[ch, h][:, bs], w_sb["wv", h][:],
                                 start=(h == 0), stop=(h == 1),
                                 skip_group_check=True)
        vT_t[g] = sr.tile([128, GC], BF16, tag="vT", bufs=6, name=f"vT{g}")
        nc.scalar.copy(vT_t[g][:, :256], vp[:, :256])
        nc.vector.tensor_copy(vT_t[g][:, 256:], vp[:, 256:])

    def stage_b(g):
        sc = pp.tile([128, 256], F32, tag="sc", bufs=PB[1], name=f"sc{g}")
        for p in range(4):
            for u in range(2):
                lh = 2 * p + u
                s64 = slice(lh * 64, (lh + 1) * 64)
                nc.tensor.matmul(sc[u * 64:(u + 1) * 64, p * 64:(p + 1) * 64],
                                 q_t[g][:, s64], k_t[g][:, s64],
                                 start=True, stop=True, skip_group_check=True)
        aT = sr.tile([128, 256], BF16, tag="aT", bufs=4, name=f"aT{g}")
        nc.scalar.activation(aT[:], sc[:], AF.Exp, scale=SCALE)
        den = sr.tile([128, 4], F32, tag="den", name=f"den{g}")
        nc.vector.reduce_sum(out=den[:],
                             in_=aT[:].rearrange("p (f i) -> p f i", i=64),
                             axis=mybir.AxisListType.X)
        rcp = sr.tile([128, 4], F32, tag="rcp", name=f"rcp{g}")
        nc.vector.reciprocal(rcp[:], den[:])
        a3 = aT[:].rearrange("p (f i) -> p f i", i=64)
        r3 = rcp[:].rearrange("p (f o) -> p f o", o=1)
        b0, b1 = broadcast_tensor_aps(a3, r3)
        nc.gpsimd.tensor_tensor(out=b0, in0=b0, in1=b1, op=MUL)
        aT_t[g] = aT

    def stage_c1(g):
        # transposes grouped by parity (avoid rapid PE tile-position toggling)
        tr = pp.tile([128, 256], BF16, tag="tr", bufs=PB[2], name=f"tr{g}")
        for u in range(2):
            rs = slice(u * 64, (u + 1) * 64)
            for p in range(4):
                nc.tensor.matmul(tr[rs, p * 64:(p + 1) * 64],
                                 aT_t[g][rs, p * 64:(p + 1) * 64],
                                 id_sb[rs, rs], is_transpose=True,
                                 start=True, stop=True, skip_group_check=True)
        # block-diagonal a per pair: [a_even 0; 0 a_odd] in a pre-zeroed tile
        aTT = sr.tile([128, 512], BF16, tag="aTT", name=f"aTT{g}")
        for u in range(2):
            rs = slice(u * 64, (u + 1) * 64)
            dst = aTT[rs, :].rearrange("p (f c) -> p f c", c=128)[
                :, :, u * 64:(u + 1) * 64]
            src = tr[rs, :].rearrange("p (f c) -> p f c", c=64)
            nc.vector.tensor_copy(dst, src)
        aTT_t[g] = aTT

    def stage_c2(g):
        # att: one full-128-contract matmul per pair, tile_position (0,0)
        at = pp.tile([128, GC], F32, tag="at", bufs=PB[3], name=f"at{g}")
        for p in range(4):
            nc.tensor.matmul(at[:, p * 128:(p + 1) * 128],
                             vT_t[g][:, p * 128:(p + 1) * 128],
                             aTT_t[g][:, p * 128:(p + 1) * 128],
                             start=True, stop=True, skip_group_check=True)
        att_t[g] = sr.tile([128, GC], BF16, tag="att", name=f"att{g}")
        nc.scalar.activation(att_t[g][:], at[:], AF.Identity, bias=bv_sb[:],
                             scale=1.0)

    def stage_d(g):
        ch, off = g // 4, (g % 4) * GC
        sl = slice(off, off + GC)
        if g % 4 == 0:
            for h in range(2):
                out_t[ch, h] = orb.tile([128, CH], BF16, tag=f"o{h}",
                                        name=f"o{h}_{ch}")
        for h in range(2):
            op = pp.tile([128, GC], F32, tag="op", bufs=PB[4], name=f"op{g}{h}")
            nc.tensor.matmul(op[:], woT_sb[:, h * 128:(h + 1) * 128],
                             att_t[g][:], start=True, stop=True)
            nc.vector.scalar_tensor_tensor(
                out_t[ch, h][:, sl], op[:], bo_sb[h][:], xb_t[ch, h][:, sl],
                op0=mybir.AluOpType.add, op1=mybir.AluOpType.add)
        if g % 4 == 3:
            for h in range(2):
                nc.gpsimd.dma_start(out_d[h * 128:(h + 1) * 128,
                                          ch * CH:(ch + 1) * CH],
                                    out_t[ch, h][:])

    # ---- software-pipelined emission ------------------------------------
    LB, L1, L2, LD = CFG["lags"]
    bis = CFG["bisect"]
    dma_in(0)
    dma_in(1)
    for s in range(NG + LD + 1):
        if s < NG:
            if s % 4 == 0:
                if s // 4 + 2 < NCH:
                    dma_in(s // 4 + 2)
        if LD <= s < NG + LD:
            stage_d(s - LD)
        if L2 <= s < NG + L2 and bis >= 3:
            stage_c2(s - L2)
        if L1 <= s < NG + L1 and bis >= 3:
            stage_c1(s - L1)
        if LB <= s < NG + LB and bis >= 2:
            stage_b(s - LB)
            if bis == 2:
                att_t[s - LB] = q_t[s - LB]
        if s < NG:
            stage_a(s)
            if bis == 1:
                att_t[s] = k_t[s]


def build():
    key = tuple(sorted((k, v) for k, v in CFG.items() if k != "trace"))
    if key in _CACHE:
        return _CACHE[key]
    nc = bacc.Bacc("TRN2", target_bir_lowering=False, debug=False,
                   num_devices=8)
    xb_d = nc.dram_tensor("xb", [C, NPOS], BF16, kind="ExternalInput")
    wkT = nc.dram_tensor("wkT", [C, S], BF16, kind="ExternalInput")
    wqT = nc.dram_tensor("wqT", [C, S], BF16, kind="ExternalInput")
    wvT = nc.dram_tensor("wvT", [C, S], BF16, kind="ExternalInput")
    woT = nc.dram_tensor("woT", [S, C], BF16, kind="ExternalInput")
    bq_d = nc.dram_tensor("bq", [S, 1], F32, kind="ExternalInput")
    bv_d = nc.dram_tensor("bv", [S, 1], F32, kind="ExternalInput")
    bo_d = nc.dram_tensor("bo", [C, 1], F32, kind="ExternalInput")
    ident = nc.dram_tensor("ident", [128, 128], BF16, kind="ExternalInput")
    out_d = nc.dram_tensor("out", [C, NPOS], BF16, kind="ExternalOutput")
    from contextlib import ExitStack
    with tile.TileContext(nc) as tc, ExitStack() as ctx:
        _emit(nc, tc, (xb_d, wkT, wqT, wvT, woT, bq_d, bv_d, bo_d, ident,
                       out_d), ctx)
    nc.compile()
    _CACHE[key] = nc
    return nc


def make_in_maps(x, wk, bk, wq, bq, wv, bv, wo, bo):
    bf = ml_dtypes.bfloat16
    x = np.asarray(x, dtype=np.float32).reshape(B, C, D, HW)
    x2 = np.ascontiguousarray(x.transpose(0, 1, 3, 2)).reshape(B, C, NPOS)
    bo = np.asarray(bo, np.float32)
    com = {
        "wkT": np.ascontiguousarray(np.asarray(wk, np.float32).T).astype(bf),
        "wqT": np.ascontiguousarray(np.asarray(wq, np.float32).T).astype(bf),
        "wvT": np.ascontiguousarray(np.asarray(wv, np.float32).T).astype(bf),
        "woT": np.ascontiguousarray(np.asarray(wo, np.float32).T).astype(bf),
        "bq": np.asarray(bq, np.float32).reshape(S, 1),
        "bv": np.asarray(bv, np.float32).reshape(S, 1),
        "bo": bo.reshape(C, 1),
        "ident": np.eye(128, dtype=bf),
    }
    return [dict(com, xb=x2[b].astype(bf)) for b in range(B)]


def postprocess(raw):
    """[B?, C, NPOS] hw-major bf16 -> [B, C, D, H, W] f32."""
    a = np.asarray(raw).astype(np.float32).reshape(-1, C, HW, D)
    return np.ascontiguousarray(a.transpose(0, 1, 3, 2)).reshape(-1, C, D, H, W)


def run(x, wk, bk, wq, bq, wv, bv, wo, bo, **kw):
    nc = build()
    maps = make_in_maps(x, wk, bk, wq, bq, wv, bv, wo, bo)
    res = run_bass_kernel_spmd(nc, maps, core_ids=list(range(B)), **kw)
    out = np.stack([np.asarray(r["out"]) for r in res.results])
    return postprocess(out), res


def kernel(x, wk, bk, wq, bq, wv, bv, wo, bo):
    out, _ = run(x, wk, bk, wq, bq, wv, bv, wo, bo)
    return out


# revision 24
# speedup vs baseline: 1.1660x; 1.0032x over previous
"""Trainium2 Bass kernel for nn_Attention_layer (dense_transformer).

One batch element per NeuronCore (8 cores).  Everything is laid out
hw-major: pos2 = hw*64 + d, so each hw owns 64 contiguous columns and the
whole kernel becomes a software-pipelined stream of 32 groups x 8 hw.

Per group g (8 hw, 512 cols), with pair p = (hw, hw+1), parity u = hw%2:
  A: k = wk@x, q = wq@x + bq          ([128,512] psum f32 -> bf16 sbuf)
     vT[pos2, s] = x^T @ wv^T          (transposed projection: no PE transpose
                                        of v needed; bv deferred to att copy)
  B: scoresT[j,i] = q_hw^T k_hw        (per hw, [64,64] at rows u*64)
     aT = exp(scale*scoresT)  (ACT), denom = row-sum (DVE), rcp (DVE),
     aT *= rcp  (Pool, stride-0 broadcast in1)
  C: a = PE-transpose(aT) per parity   (bf16 psum)
     att[s, j] = vT_hw^T @ a_hw        (+ bv via ACT copy; sum_i a = 1)
  D: out = (woT^T @ att + bo) + x via one DVE scalar_tensor_tensor per
     half (bias and residual fused), bf16 out, DMA per 2048-col chunk.

Bias algebra: bk drops (constant in softmax axis), bq folds into q copy,
bv into att copy (softmax rows sum to 1), bo into the residual STT op.
"""

import numpy as np
import ml_dtypes

import concourse.bacc as bacc
import concourse.tile as tile
from concourse import mybir
from concourse.bass import broadcast_tensor_aps
from concourse.bass_utils import run_bass_kernel_spmd

F32 = mybir.dt.float32
BF16 = mybir.dt.bfloat16
AF = mybir.ActivationFunctionType
MUL = mybir.AluOpType.mult

B, C, S, D, H, W = 8, 256, 128, 64, 16, 16
HW = H * W              # 256
NPOS = D * HW           # 16384 (hw-major: pos2 = hw*64 + d)
SCALE = float(1.0 / np.sqrt(np.float32(S)))

NG = 32                 # groups
GHW = 8                 # hw per group
GC = GHW * D            # 512 cols per group
CH = 2048               # dma chunk cols (4 groups)
NCH = NPOS // CH        # 8

CFG = {
    "loop_n": 1,
    "lags": (2, 3, 4, 5),  # stage lags (B, C1, C2, D) behind A
    "pbufs": (2, 1, 1, 2, 2),  # psum ring depths (pj, sc, tr, at, op)
    "bisect": 3,        # 1: A+D only, 2: A+B+D, 3: full
    "trace": False,
}

_CACHE = {}


def _emit(nc, tc, io, ctx):
    xb_d, wkT, wqT, wvT, woT, bq_d, bv_d, bo_d, ident, out_d = io
    PB = CFG["pbufs"]

    const = ctx.enter_context(tc.tile_pool(name="const", bufs=1))
    xr = ctx.enter_context(tc.tile_pool(name="xr", bufs=3))
    sr = ctx.enter_context(tc.tile_pool(name="sr", bufs=3))
    orb = ctx.enter_context(tc.tile_pool(name="orb", bufs=2))
    pp = ctx.enter_context(tc.tile_pool(name="pp", bufs=2, space="PSUM"))

    # ---- constants ------------------------------------------------------
    id_sb = const.tile([128, 128], BF16, tag="ident")
    nc.sync.dma_start(id_sb[:], ident[:])
    w_sb = {}
    for nm, t in (("wk", wkT), ("wq", wqT), ("wv", wvT)):
        for h in range(2):
            w_sb[nm, h] = const.tile([128, 128], BF16, tag=f"w_{nm}{h}",
                                     name=f"w_{nm}{h}")
            nc.sync.dma_start(w_sb[nm, h][:], t[h * 128:(h + 1) * 128, :])
    woT_sb = const.tile([128, 256], BF16, tag="woT")
    nc.sync.dma_start(woT_sb[:], woT[:])
    bq_sb = const.tile([128, 1], F32, tag="bq")
    nc.sync.dma_start(bq_sb[:], bq_d[:])
    bv_sb = const.tile([128, 1], F32, tag="bv")
    nc.sync.dma_start(bv_sb[:], bv_d[:])
    bo_sb = {}
    for h in range(2):
        bo_sb[h] = const.tile([128, 1], F32, tag=f"bo{h}", name=f"bo{h}")
        nc.sync.dma_start(bo_sb[h][:], bo_d[h * 128:(h + 1) * 128, :])

    # pre-zero the aTT ring slots once: in-loop writes only touch the
    # diagonal quadrants, off-diagonal zeros persist across groups/iters
    for z in range(3):
        zt = sr.tile([128, 512], BF16, tag="aTT", name=f"aTTz{z}")
        nc.gpsimd.memset(zt[:], 0.0)

    loop_cm = tc.For_i(0, CFG["loop_n"], 1) if CFG["loop_n"] > 1 else None
    if loop_cm is not None:
        ctx.enter_context(loop_cm)

    xb_t = {}
    k_t, q_t, vT_t, aT_t, aTT_t, att_t = {}, {}, {}, {}, {}, {}
    rcp_t, out_t = {}, {}

    def dma_in(ch):
        for h in range(2):
            t = xr.tile([128, CH], BF16, tag=f"xb{h}", bufs=6, name=f"xb{h}_{ch}")
            nc.sync.dma_start(t[:], xb_d[h * 128:(h + 1) * 128,
                                         ch * CH:(ch + 1) * CH])
            xb_t[ch, h] = t

    def stage_a(g):
        ch, off = g // 4, (g % 4) * GC
        sl = slice(off, off + GC)
        kp = pp.tile([128, GC], F32, tag="pj", bufs=PB[0], name=f"kp{g}")
        nc.tensor.matmul(kp[:], w_sb["wk", 0][:], xb_t[ch, 0][:, sl],
                         start=True, stop=False)
        nc.tensor.matmul(kp[:], w_sb["wk", 1][:], xb_t[ch, 1][:, sl],
                         start=False, stop=True)
        k_t[g] = sr.tile([128, GC], BF16, tag="k", bufs=6, name=f"k{g}")
        nc.scalar.copy(k_t[g][:], kp[:])
        qp = pp.tile([128, GC], F32, tag="pj", bufs=PB[0], name=f"qp{g}")
        nc.tensor.matmul(qp[:], w_sb["wq", 0][:], xb_t[ch, 0][:, sl],
                         start=True, stop=False)
        nc.tensor.matmul(qp[:], w_sb["wq", 1][:], xb_t[ch, 1][:, sl],
                         start=False, stop=True)
        q_t[g] = sr.tile([128, GC], BF16, tag="q", bufs=6, name=f"q{g}")
        nc.scalar.activation(q_t[g][:], qp[:], AF.Identity, bias=bq_sb[:],
                             scale=1.0)
        vp = pp.tile([128, GC], F32, tag="pj", bufs=PB[0], name=f"vp{g}")
        for p in range(4):
            bs = slice(off + p * 128, off + (p + 1) * 128)
            for h in range(2):
                nc.tensor.matmul(vp[:, p * 128:(p + 1) * 128],
                                 xb_t[ch, h][:, bs], w_sb["wv", h][:],
                                 start=(h == 0), stop=(h == 1),
                                 skip_group_check=True)
        vT_t[g] = sr.tile([128, GC], BF16, tag="vT", bufs=8, name=f"vT{g}")
        nc.scalar.copy(vT_t[g][:, :256], vp[:, :256])
        nc.vector.tensor_copy(vT_t[g][:, 256:], vp[:, 256:])

    def stage_b(g):
        sc = pp.tile([128, 256], F32, tag="sc", bufs=PB[1], name=f"sc{g}")
        for p in range(4):
            for u in range(2):
                lh = 2 * p + u
                s64 = slice(lh * 64, (lh + 1) * 64)
                nc.tensor.matmul(sc[u * 64:(u + 1) * 64, p * 64:(p + 1) * 64],
                                 q_t[g][:, s64], k_t[g][:, s64],
                                 start=True, stop=True, skip_group_check=True)
        aT = sr.tile([128, 256], BF16, tag="aT", bufs=6, name=f"aT{g}")
        nc.scalar.activation(aT[:], sc[:], AF.Exp, scale=SCALE)
        den = sr.tile([128, 4], F32, tag="den", name=f"den{g}")
        nc.vector.reduce_sum(out=den[:],
                             in_=aT[:].rearrange("p (f i) -> p f i", i=64),
                             axis=mybir.AxisListType.X)
        rcp = sr.tile([128, 4], F32, tag="rcp", name=f"rcp{g}")
        nc.vector.reciprocal(rcp[:], den[:])
        a3 = aT[:].rearrange("p (f i) -> p f i", i=64)
        r3 = rcp[:].rearrange("p (f o) -> p f o", o=1)
        b0, b1 = broadcast_tensor_aps(a3, r3)
        nc.gpsimd.tensor_tensor(out=b0, in0=b0, in1=b1, op=MUL)
        aT_t[g] = aT

    def stage_c1(g):
        # transposes grouped by parity (avoid rapid PE tile-position toggling)
        tr = pp.tile([128, 256], BF16, tag="tr", bufs=PB[2], name=f"tr{g}")
        for u in range(2):
            rs = slice(u * 64, (u + 1) * 64)
            for p in range(4):
                nc.tensor.matmul(tr[rs, p * 64:(p + 1) * 64],
                                 aT_t[g][rs, p * 64:(p + 1) * 64],
                                 id_sb[rs, rs], is_transpose=True,
                                 start=True, stop=True, skip_group_check=True)
        # block-diagonal a per pair: [a_even 0; 0 a_odd] in a pre-zeroed tile
        aTT = sr.tile([128, 512], BF16, tag="aTT", name=f"aTT{g}")
        for u in range(2):
            rs = slice(u * 64, (u + 1) * 64)
            dst = aTT[rs, :].rearrange("p (f c) -> p f c", c=128)[
                :, :, u * 64:(u + 1) * 64]
            src = tr[rs, :].rearrange("p (f c) -> p f c", c=64)
            nc.vector.tensor_copy(dst, src)
        aTT_t[g] = aTT

    def stage_c2(g):
        # att: one full-128-contract matmul per pair, tile_position (0,0)
        at = pp.tile([128, GC], F32, tag="at", bufs=PB[3], name=f"at{g}")
        for p in range(4):
            nc.tensor.matmul(at[:, p * 128:(p + 1) * 128],
                             vT_t[g][:, p * 128:(p + 1) * 128],
                             aTT_t[g][:, p * 128:(p + 1) * 128],
                             start=True, stop=True, skip_group_check=True)
        att_t[g] = sr.tile([128, GC], BF16, tag="att", bufs=5, name=f"att{g}")
        nc.scalar.activation(att_t[g][:], at[:], AF.Identity, bias=bv_sb[:],
                             scale=1.0)

    def stage_d(g):
        ch, off = g // 4, (g % 4) * GC
        sl = slice(off, off + GC)
        if g % 4 == 0:
            for h in range(2):
                out_t[ch, h] = orb.tile([128, CH], BF16, tag=f"o{h}",
                                        name=f"o{h}_{ch}")
        for h in range(2):
            op = pp.tile([128, GC], F32, tag="op", bufs=PB[4], name=f"op{g}{h}")
            nc.tensor.matmul(op[:], woT_sb[:, h * 128:(h + 1) * 128],
                             att_t[g][:], start=True, stop=True)
            nc.vector.scalar_tensor_tensor(
                out_t[ch, h][:, sl], op[:], bo_sb[h][:], xb_t[ch, h][:, sl],
                op0=mybir.AluOpType.add, op1=mybir.AluOpType.add)
        if g % 4 == 3:
            for h in range(2):
                nc.gpsimd.dma_start(out_d[h * 128:(h + 1) * 128,
                                          ch * CH:(ch + 1) * CH],
                                    out_t[ch, h][:])

    # ---- software-pipelined emission ------------------------------------
    LB, L1, L2, LD = CFG["lags"]
    bis = CFG["bisect"]
    dma_in(0)
    dma_in(1)
    for s in range(NG + LD + 1):
        if s < NG:
            if s % 4 == 0:
                if s // 4 + 2 < NCH:
                    dma_in(s // 4 + 2)
        if LD <= s < NG + LD:
            stage_d(s - LD)
        if L2 <= s < NG + L2 and bis >= 3:
            stage_c2(s - L2)
        if L1 <= s < NG + L1 and bis >= 3:
            stage_c1(s - L1)
        if LB <= s < NG + LB and bis >= 2:
            stage_b(s - LB)
            if bis == 2:
                att_t[s - LB] = q_t[s - LB]
        if s < NG:
            stage_a(s)
            if bis == 1:
                att_t[s] = k_t[s]


def build():
    key = tuple(sorted((k, v) for k, v in CFG.items() if k != "trace"))
    if key in _CACHE:
        return _CACHE[key]
    nc = bacc.Bacc("TRN2", target_bir_lowering=False, debug=False,
                   num_devices=8)
    xb_d = nc.dram_tensor("xb", [C, NPOS], BF16, kind="ExternalInput")
    wkT = nc.dram_tensor("wkT", [C, S], BF16, kind="ExternalInput")
    wqT = nc.dram_tensor("wqT", [C, S], BF16, kind="ExternalInput")
    wvT = nc.dram_tensor("wvT", [C, S], BF16, kind="ExternalInput")
    woT = nc.dram_tensor("woT", [S, C], BF16, kind="ExternalInput")
    bq_d = nc.dram_tensor("bq", [S, 1], F32, kind="ExternalInput")
    bv_d = nc.dram_tensor("bv", [S, 1], F32, kind="ExternalInput")
    bo_d = nc.dram_tensor("bo", [C, 1], F32, kind="ExternalInput")
    ident = nc.dram_tensor("ident", [128, 128], BF16, kind="ExternalInput")
    out_d = nc.dram_tensor("out", [C, NPOS], BF16, kind="ExternalOutput")
    from contextlib import ExitStack
    with tile.TileContext(nc) as tc, ExitStack() as ctx:
        _emit(nc, tc, (xb_d, wkT, wqT, wvT, woT, bq_d, bv_d, bo_d, ident,
                       out_d), ctx)
    nc.compile()
    _CACHE[key] = nc
    return nc


def make_in_maps(x, wk, bk, wq, bq, wv, bv, wo, bo):
    bf = ml_dtypes.bfloat16
    x = np.asarray(x, dtype=np.float32).reshape(B, C, D, HW)
    x2 = np.ascontiguousarray(x.transpose(0, 1, 3, 2)).reshape(B, C, NPOS)
    bo = np.asarray(bo, np.float32)
    com = {
        "wkT": np.ascontiguousarray(np.asarray(wk, np.float32).T).astype(bf),
        "wqT": np.ascontiguousarray(np.asarray(wq, np.float32).T).astype(bf),
        "wvT": np.ascontiguousarray(np.asarray(wv, np.float32).T).astype(bf),
        "woT": np.ascontiguousarray(np.asarray(wo, np.float32).T).astype(bf),
        "bq": np.asarray(bq, np.float32).reshape(S, 1),
        "bv": np.asarray(bv, np.float32).reshape(S, 1),
        "bo": bo.reshape(C, 1),
        "ident": np.eye(128, dtype=bf),
    }
    return [dict(com, xb=x2[b].astype(bf)) for b in range(B)]


def postprocess(raw):
    """[B?, C, NPOS] hw-major bf16 -> [B, C, D, H, W] f32."""
    a = np.asarray(raw).astype(np.float32).reshape(-1, C, HW, D)
    return np.ascontiguousarray(a.transpose(0, 1, 3, 2)).reshape(-1, C, D, H, W)


def run(x, wk, bk, wq, bq, wv, bv, wo, bo, **kw):
    nc = build()
    maps = make_in_maps(x, wk, bk, wq, bq, wv, bv, wo, bo)
    res = run_bass_kernel_spmd(nc, maps, core_ids=list(range(B)), **kw)
    out = np.stack([np.asarray(r["out"]) for r in res.results])
    return postprocess(out), res


def kernel(x, wk, bk, wq, bq, wv, bv, wo, bo):
    out, _ = run(x, wk, bk, wq, bq, wv, bv, wo, bo)
    return out
